# revision 1
# baseline (speedup 1.0000x reference)
"""Trainium2 Bass kernel for nn_ExpertsMLPBlock (MoE routing) — v3.

Problem (hardcoded):
  x          [8, 4096, 256] f32
  rms_weight [256]          f32
  W1         [8, 256, 1024] f32   b1 [8, 1024] f32
  W2         [8, 1024, 256] f32   b2 [8, 256]  f32
  expert_ids [8, 4096, 2]   int   (values 0..7)
  out        [8, 4096, 2, 256] f32

Sharding: data-parallel over B (batch row b -> core b), expert weights
replicated (pre-cast to bf16 on host, rms_weight folded into W1).

Per-core algorithm:
  A. RMSNorm tokens -> xn (bf16) in DRAM (squares + row sums on Act engine).
  B. Routing on DVE/PE: for slot (p,c) = (token (c//2)*128+p, k=c%2) compute
     wrap-row w = e*CAP + (r%16)*72 + r//16 (r = rank of slot within expert,
     slot order c-major); capacity overflow clamped to dump row NROW.
  T. Rank table tbl[NROWT, 128]i16 (256B rows, zero-init): row w gets the
     single value v = 2*token + k + 1 via 8 chunked dma_scatter_adds whose
     wrap-16 idxs (i = p*64+c) come from a DRAM bounce + partition-shift
     replication.  Then extract column 0 -> compact src16_d[NROWT] (bulk
     SBUF load + DVE + contiguous store).  Because rows are wrap-ordered,
     per-expert idx lists (partition r%16, col r//16) are contiguous runs.
  D. Per expert e (pipelined):
       idx list: idxt = ((v-1)>>1)&4095  (padding v=0 -> garbage token)
       dma_gather(transpose=True): xgT[128, 2, CAP] bf16 straight from xn
       h^T = gelu(W1^T xn^T + b1); y = h W2 + b2 -> yE[128, 9, 256] f32
       dst rows: v-1 (padding -> OOB via +1e5 trick); 9 indirect scatters
       write y rows straight to out[t, k, :].
"""

import numpy as np


import concourse.bacc as bacc
import concourse.bass as bass
import concourse.mybir as mybir
from concourse import bass_utils
from concourse.tile import TileContext
from concourse.alu_op_type import AluOpType

F32 = mybir.dt.float32
BF16 = mybir.dt.bfloat16
I32 = mybir.dt.int32
I16 = mybir.dt.int16

B, T, C, WH, E, K = 8, 4096, 256, 1024, 8, 2
NSLOT = T * K          # 8192 slots per core
NCOL = 64              # slot (p, c)
CAP = 1152             # per-expert capacity; max observed count ~1100
NBLK = CAP // 128      # 9
SW = CAP // 16         # 72 wrap cols per expert
NROW = E * CAP         # 9216
NROWT = NROW + 128     # + dump rows
TW = 128               # table row width in i16 (256B)
NCH = 2                # scatter_add chunks
RMS_EPS = 1.1920928955078125e-07
ACT_GELU = mybir.ActivationFunctionType.Gelu
ACT_SQRT = mybir.ActivationFunctionType.Sqrt
ACT_SQUARE = mybir.ActivationFunctionType.Square
ACT_COPY = mybir.ActivationFunctionType.Copy

_CACHE = {}


def _build():
    nc = bacc.Bacc("TRN2", target_bir_lowering=False, debug=False, num_devices=8)

    x_d = nc.dram_tensor("x", [T, C], F32, kind="ExternalInput")
    w1_d = nc.dram_tensor("w1", [128, E * 2 * WH], BF16, kind="ExternalInput")
    w2_d = nc.dram_tensor("w2", [128, E * 8 * C], BF16, kind="ExternalInput")
    b1_d = nc.dram_tensor("b1", [128, E * 8], F32, kind="ExternalInput")
    b2_d = nc.dram_tensor("b2", [E, C], F32, kind="ExternalInput")
    ids_d = nc.dram_tensor("ids", [128, NCOL], F32, kind="ExternalInput")
    iotae_d = nc.dram_tensor("iotae", [128, 8], F32, kind="ExternalInput")
    u128_d = nc.dram_tensor("u128", [128, 128], F32, kind="ExternalInput")
    ones128_d = nc.dram_tensor("ones128", [128, 128], F32, kind="ExternalInput")
    vals_d = nc.dram_tensor("vals", [128, NCOL], I16, kind="ExternalInput")
    iotar_d = nc.dram_tensor("iotar", [128, NBLK], F32, kind="ExternalInput")
    out_d = nc.dram_tensor("out", [T, K, C], F32, kind="ExternalOutput")
    out_flat = out_d.ap().rearrange("t k c -> (t k) c")

    with TileContext(nc) as tc:
        with (
            tc.tile_pool(name="const", bufs=1) as constp,
            tc.tile_pool(name="norm", bufs=2) as normp,
            tc.tile_pool(name="route", bufs=1) as routep,
            tc.tile_pool(name="slab", bufs=1) as slabp,
            tc.tile_pool(name="act", bufs=4) as actp,
            tc.tile_pool(name="hpool", bufs=3) as hp,
            tc.tile_pool(name="ypool", bufs=2) as yp,
            tc.tile_pool(name="psum", bufs=4, space="PSUM") as pp,
            tc.tile_pool(name="psumy", bufs=2, space="PSUM") as ppy,
            tc.tile_pool(name="psumt", bufs=1, space="PSUM") as ppt,
            tc.tile_pool(name="dram", bufs=1, space="DRAM") as dp,
        ):
            # ---- DRAM staging ----
            xn_dram = dp.tile([T, C], BF16)
            tbl = dp.tile([NROWT, TW], I16)
            tmpw = dp.tile([NSLOT], I16)
            src16_d = dp.tile([NROWT], I16)

            # ---- constants / weights (prefetch at t=0) ----
            idst = constp.tile([128, NCOL], F32)
            nc.sync.dma_start(out=idst[:], in_=ids_d[:])
            iotae = constp.tile([128, 8], F32)
            nc.scalar.dma_start(out=iotae[:], in_=iotae_d[:])
            u128 = constp.tile([128, 128], F32)
            nc.scalar.dma_start(out=u128[:], in_=u128_d[:])
            ones128 = constp.tile([128, 128], F32)
            nc.scalar.dma_start(out=ones128[:], in_=ones128_d[:])
            vals16 = constp.tile([128, NCOL], I16)
            nc.scalar.dma_start(out=vals16[:], in_=vals_d[:])
            iotar = constp.tile([128, NBLK], F32)
            nc.scalar.dma_start(out=iotar[:], in_=iotar_d[:])
            ztbl = constp.tile([128, TW], I16)
            nc.vector.memset(ztbl[:], 0.0)
            nc.scalar.dma_start(
                out=tbl[:].rearrange("(p a) w -> p a w", p=128),
                in_=ztbl[:].rearrange("p (o w) -> p o w", o=1).to_broadcast(
                    [128, NROWT // 128, TW]
                ),
            )
            w1sb = constp.tile([128, E, 2, WH], BF16)
            nc.sync.dma_start(
                out=w1sb[:], in_=w1_d.ap().rearrange("p (e c w) -> p e c w", e=E, c=2)
            )
            w2sb = constp.tile([128, E, 8, C], BF16)
            nc.scalar.dma_start(
                out=w2sb[:], in_=w2_d.ap().rearrange("p (e w c) -> p e w c", e=E, w=8)
            )
            b1sb = constp.tile([128, E, 8], F32)
            nc.scalar.dma_start(
                out=b1sb[:], in_=b1_d.ap().rearrange("p (e w) -> p e w", e=E)
            )
            b2sb = constp.tile([128, E, C], F32)
            nc.scalar.dma_start(
                out=b2sb[:],
                in_=b2_d.ap().rearrange("(o e) c -> o e c", o=1).to_broadcast([128, E, C]),
            )
            # scatter_add value chunks (zero rows, v at elem 0) - no deps
            tvcs = []
            for ch in range(NCH):
                CHW = NSLOT // NCH // 128
                tvc = slabp.tile([128, CHW, TW], I16, tag=f"tvc{ch}")
                nc.vector.memset(tvc[:].rearrange("p a w -> p (a w)"), 0.0)
                nc.vector.tensor_copy(
                    tvc[:, :, 0], vals16[:, ch * CHW:(ch + 1) * CHW]
                )
                tvcs.append(tvc)

            # ---- phase B: routing -> wrap-row offsets ----
            oh = routep.tile([128, NCOL, 8], F32, tag="oh")
            nc.vector.tensor_tensor(
                out=oh[:],
                in0=idst[:].rearrange("p (c o) -> p c o", o=1).to_broadcast([128, NCOL, 8]),
                in1=iotae[:].rearrange("p (o e) -> p o e", o=1).to_broadcast([128, NCOL, 8]),
                op=AluOpType.is_equal,
            )
            ohf = oh[:].rearrange("p c e -> p (c e)")           # [128, 512]
            PAD = 256
            sc = [routep.tile([128, PAD + NCOL * 8], F32, tag=f"sc{i}", name=f"sc{i}") for i in range(2)]
            nc.vector.memset(sc[0][:, :PAD], 0.0)
            nc.vector.memset(sc[1][:, :PAD], 0.0)
            nc.vector.tensor_copy(sc[0][:, PAD:], ohf)
            cur = sc[0][:]
            for i, s in enumerate([8, 16, 32, 64, 128, 256]):
                nxt = sc[(i + 1) % 2][:]
                nc.vector.tensor_add(
                    nxt[:, PAD:], cur[:, PAD:], cur[:, PAD - s:PAD + 512 - s]
                )
                cur = nxt
            cur = cur[:, PAD:]
            rk = ppt.tile([128, NCOL * 8], F32, tag="rk")
            nc.tensor.matmul(rk[:], lhsT=u128[:], rhs=ohf, start=True, stop=False)
            nc.tensor.matmul(
                rk[:, 8:], lhsT=ones128[:], rhs=cur[:, :504], start=False, stop=True
            )
            prod = routep.tile([128, NCOL * 8], F32, tag="prod")
            nc.vector.tensor_mul(prod[:], rk[:], ohf)
            p4 = prod[:].rearrange("p (ce two) -> p ce two", two=2)
            f1 = routep.tile([128, NCOL * 4], F32, tag="f1")
            nc.vector.tensor_add(f1[:], p4[:, :, 0], p4[:, :, 1])
            f4 = f1[:].rearrange("p (ce two) -> p ce two", two=2)
            f2 = routep.tile([128, NCOL * 2], F32, tag="f2")
            nc.vector.tensor_add(f2[:], f4[:, :, 0], f4[:, :, 1])
            f5 = f2[:].rearrange("p (ce two) -> p ce two", two=2)
            sel = routep.tile([128, NCOL], F32, tag="sel")  # rank+1
            nc.vector.tensor_add(sel[:], f5[:, :, 0], f5[:, :, 1])
            # wrap-row: w = (r%16)*71.9375... -> rm*71.9375 + r*0.0625 + e*CAP
            rank0 = routep.tile([128, NCOL], F32, tag="rank0")
            nc.vector.tensor_scalar(
                out=rank0[:], in0=sel[:], scalar1=-1.0, scalar2=0.0,
                op0=AluOpType.add, op1=AluOpType.add,
            )
            r32 = routep.tile([128, NCOL], I32, tag="r32")
            nc.vector.tensor_copy(r32[:], rank0[:])
            rd32 = routep.tile([128, NCOL], I32, tag="rd32")
            nc.vector.tensor_scalar(
                out=rd32[:], in0=r32[:], scalar1=4, scalar2=0x7FFFFFFF,
                op0=AluOpType.logical_shift_right, op1=AluOpType.bitwise_and,
            )
            rm32 = routep.tile([128, NCOL], I32, tag="rm32")
            nc.vector.tensor_scalar(
                out=rm32[:], in0=r32[:], scalar1=15, scalar2=0x7FFFFFFF,
                op0=AluOpType.bitwise_and, op1=AluOpType.bitwise_and,
            )
            rmf = routep.tile([128, NCOL], F32, tag="rmf")
            nc.vector.tensor_scalar(
                out=rmf[:], in0=rm32[:], scalar1=72, scalar2=0,
                op0=AluOpType.mult, op1=AluOpType.add,
            )
            rdf = routep.tile([128, NCOL], F32, tag="rdf")
            nc.vector.tensor_copy(rdf[:], rd32[:])
            w0 = routep.tile([128, NCOL], F32, tag="w0")
            nc.vector.tensor_add(w0[:], rmf[:], rdf[:])
            ecap = routep.tile([128, NCOL], F32, tag="ecap")
            nc.vector.tensor_scalar(
                out=ecap[:], in0=idst[:], scalar1=float(CAP), scalar2=0.0,
                op0=AluOpType.mult, op1=AluOpType.add,
            )
            w1r = routep.tile([128, NCOL], F32, tag="w1r")
            nc.vector.tensor_add(w1r[:], w0[:], ecap[:])
            penal = routep.tile([128, NCOL], F32, tag="penal")
            nc.vector.tensor_scalar(
                out=penal[:], in0=sel[:], scalar1=float(CAP), scalar2=1.0e6,
                op0=AluOpType.is_gt, op1=AluOpType.mult,
            )
            w2r = routep.tile([128, NCOL], F32, tag="w2r")
            nc.vector.tensor_add(w2r[:], w1r[:], penal[:])
            offt = routep.tile([128, NCOL], F32, tag="offt")
            nc.vector.tensor_scalar(
                out=offt[:], in0=w2r[:], scalar1=float(NROW), scalar2=0.0,
                op0=AluOpType.min, op1=AluOpType.add,
            )
            offi32 = routep.tile([128, NCOL], I32, tag="offi32")
            nc.vector.tensor_copy(offi32[:], offt[:])
            offi16 = routep.tile([128, NCOL], I16, tag="offi16")
            nc.vector.tensor_copy(offi16[:], offi32[:])
            # per-expert totals -> every partition (ones128 @ colsums), then
            # validity mask[p, e, b] = (count_e > b*128 + p)
            cntp = ppt.tile([128, 8], F32, tag="cntp")
            nc.tensor.matmul(
                cntp[:], lhsT=ones128[:], rhs=cur[:, 504:512], start=True, stop=True
            )
            mask = constp.tile([128, E, NBLK], F32)
            nc.vector.tensor_tensor(
                out=mask[:],
                in0=cntp[:].rearrange("p (e o) -> p e o", o=1).to_broadcast(
                    [128, E, NBLK]
                ),
                in1=iotar[:].rearrange("p (o b) -> p o b", o=1).to_broadcast(
                    [128, E, NBLK]
                ),
                op=AluOpType.is_gt,
            )

            # ---- phase T: wrap-16 idxs (i = p*64+c) via bounce; scatter_adds ----
            nc.sync.dma_start(
                out=tmpw[:].rearrange("(p c) -> p c", p=128), in_=offi16[:]
            )
            idxw = constp.tile([128, NSLOT // 16], I16)
            nc.sync.dma_start(
                out=idxw[0:16, :], in_=tmpw[:].rearrange("(s q) -> q s", q=16)
            )
            for rg in range(1, 8):
                nc.sync.dma_start(
                    out=idxw[rg * 16:(rg + 1) * 16, :], in_=idxw[0:16, :]
                )
            CH = NSLOT // NCH
            for ch in range(NCH):
                nc.gpsimd.dma_scatter_add(
                    out_ap=tbl[:],
                    in_ap=tvcs[ch][:],
                    idxs_ap=idxw[:, ch * (CH // 16):(ch + 1) * (CH // 16)],
                    num_idxs=CH,
                    num_idxs_reg=CH,
                    elem_size=TW,
                    single_packet=False,
                )

            # ---- phase A: RMSNorm -> xn_dram (bf16), 4 token-tiles/iter ----
            AW = 4
            for m in range(T // (128 * AW)):
                xt = normp.tile([128, AW, C], F32, tag="xt")
                nc.sync.dma_start(
                    out=xt[:],
                    in_=x_d[m * 128 * AW:(m + 1) * 128 * AW, :].rearrange(
                        "(a p) c -> p a c", p=128
                    ),
                )
                xnb = normp.tile([128, AW, C], BF16, tag="xnb")
                ms = normp.tile([128, AW], F32, tag="ms")
                for a in range(AW):
                    nc.scalar.activation(
                        xnb[:, a, :], xt[:, a, :], ACT_SQUARE,
                        accum_out=ms[:, a:a + 1],
                    )
                ms2 = normp.tile([128, AW], F32, tag="ms2")
                nc.vector.tensor_scalar(
                    out=ms2[:], in0=ms[:], scalar1=1.0 / C, scalar2=RMS_EPS,
                    op0=AluOpType.mult, op1=AluOpType.add,
                )
                sr = normp.tile([128, AW], F32, tag="sr")
                nc.scalar.activation(sr[:], ms2[:], ACT_SQRT)
                rstd = normp.tile([128, AW], F32, tag="rstd")
                nc.vector.reciprocal(rstd[:], sr[:])
                for a in range(AW):
                    nc.scalar.activation(
                        xnb[:, a, :], xt[:, a, :], ACT_COPY, scale=rstd[:, a:a + 1]
                    )
                nc.sync.dma_start(
                    out=xn_dram[m * 128 * AW:(m + 1) * 128 * AW, :].rearrange(
                        "(a p) c -> p a c", p=128
                    ),
                    in_=xnb[:],
                )

            # ---- extraction: tbl[:, 0] -> compact src16_d (2 half-bulk loads) ----
            tblview = tbl[:].rearrange("(p a) w -> p a w", p=128)
            srcview = src16_d[:].rearrange("(p a) -> p a", p=128)
            for (a0, a1) in [(0, 37), (37, 73)]:
                hb = constp.tile([128, 37, TW], I16, tag="halfbulk")
                nc.sync.dma_start(
                    out=hb[:, :a1 - a0, :], in_=tblview[:, a0:a1, :]
                )
                hc = constp.tile([128, 37], I16, tag="halfc0")
                nc.vector.tensor_copy(hc[:, :a1 - a0], hb[:, :a1 - a0, 0])
                nc.sync.dma_start(
                    out=srcview[:, a0:a1], in_=hc[:, :a1 - a0]
                )

            # gather idx lists for all experts: [128, E, 72], replicated x8
            idxg = constp.tile([128, E, SW], I16)
            nc.scalar.dma_start(
                out=idxg[0:16, :, :],
                in_=src16_d[:NROW].rearrange("(e q s) -> q e s", q=16, e=E),
            )
            for rg in range(1, 8):
                nc.scalar.dma_start(
                    out=idxg[rg * 16:(rg + 1) * 16, :, :], in_=idxg[0:16, :, :]
                )
            idxt = constp.tile([128, E, SW], I16)
            g32 = constp.tile([128, E, SW], I32)
            nc.vector.tensor_copy(
                g32[:].rearrange("p e s -> p (e s)"),
                idxg[:].rearrange("p e s -> p (e s)"),
            )
            m32 = constp.tile([128, E, SW], I32)
            nc.vector.tensor_scalar(
                out=m32[:].rearrange("p e s -> p (e s)"),
                in0=g32[:].rearrange("p e s -> p (e s)"),
                scalar1=-1, scalar2=0,
                op0=AluOpType.add, op1=AluOpType.add,
            )
            nc.vector.tensor_scalar(
                out=g32[:].rearrange("p e s -> p (e s)"),
                in0=m32[:].rearrange("p e s -> p (e s)"),
                scalar1=1, scalar2=4095,
                op0=AluOpType.logical_shift_right, op1=AluOpType.bitwise_and,
            )
            nc.vector.tensor_copy(
                idxt[:].rearrange("p e s -> p (e s)"),
                g32[:].rearrange("p e s -> p (e s)"),
            )
            # scatter_add idx list: max(v-1, 0) as int16 (padding -> row 0)
            nc.vector.tensor_scalar(
                out=g32[:].rearrange("p e s -> p (e s)"),
                in0=m32[:].rearrange("p e s -> p (e s)"),
                scalar1=0, scalar2=0,
                op0=AluOpType.max, op1=AluOpType.add,
            )
            dstw = constp.tile([128, E, SW], I16)
            nc.vector.tensor_copy(
                dstw[:].rearrange("p e s -> p (e s)"),
                g32[:].rearrange("p e s -> p (e s)"),
            )
            # zero-init output (scatter_add accumulates into it)
            zout = constp.tile([128, C], F32)
            nc.vector.memset(zout[:], 0.0)
            nc.scalar.dma_start(
                out=out_flat[0:T].rearrange("(a p) c -> p a c", p=128),
                in_=zout[:].rearrange("p (o c) -> p o c", o=1).to_broadcast(
                    [128, T // 128, C]
                ),
            )
            nc.scalar.dma_start(
                out=out_flat[T:2 * T].rearrange("(a p) c -> p a c", p=128),
                in_=zout[:].rearrange("p (o c) -> p o c", o=1).to_broadcast(
                    [128, T // 128, C]
                ),
            )


            # ---- phase D: per-expert MLP (gathers software-pipelined) ----
            t5_sizes = [512, 512, CAP - 1024]

            def issue_gather(e):
                xg = actp.tile([128, 2, CAP], BF16, tag="xgT")
                nc.gpsimd.dma_gather(
                    out_ap=xg[:],
                    in_ap=xn_dram[:],
                    idxs_ap=idxt[:, e, :],
                    num_idxs=CAP,
                    num_idxs_reg=CAP,
                    elem_size=C,
                    transpose=True,
                    single_packet=False,
                )
                return xg

            PFD = 4
            xg_tiles = [issue_gather(e) for e in range(PFD)]
            for e in range(E):
                xgT = xg_tiles[e]

                yE = yp.tile([128, NBLK, C], F32, tag="yE")
                hTs = []
                for t5 in range(3):
                    ts = t5_sizes[t5]
                    off = t5 * 512
                    hT = hp.tile([128, 8, 512], BF16, tag="hT")
                    hTs.append(hT)
                    for wc in range(8):
                        hps = pp.tile([128, 512], F32, tag="hps")
                        for cc in range(2):
                            nc.tensor.matmul(
                                hps[:, :ts],
                                lhsT=w1sb[:, e, cc, wc * 128:(wc + 1) * 128],
                                rhs=xgT[:, cc, off:off + ts],
                                start=(cc == 0), stop=(cc == 1),
                            )
                        nc.scalar.activation(
                            hT[:, wc, :ts], hps[:, :ts], ACT_GELU,
                            bias=b1sb[:, e, wc:wc + 1],
                        )
                for t5 in range(3):
                    ts = t5_sizes[t5]
                    hT = hTs[t5]
                    for tb in range(ts // 128):
                        blk = t5 * 4 + tb
                        yps = ppy.tile([128, C], F32, tag="yps")
                        for wc in range(8):
                            nc.tensor.matmul(
                                yps[:],
                                lhsT=hT[:, wc, tb * 128:(tb + 1) * 128],
                                rhs=w2sb[:, e, wc, :],
                                start=(wc == 0), stop=(wc == 7),
                            )
                        nc.vector.tensor_add(yE[:, blk, :], yps[:], b2sb[:, e, :])
                        nc.vector.tensor_tensor(
                            out=yE[:, blk, :],
                            in0=yE[:, blk, :],
                            in1=mask[:, e, blk:blk + 1].to_broadcast([128, C]),
                            op=AluOpType.mult,
                        )

                if e + PFD < E:
                    xg_tiles.append(issue_gather(e + PFD))
                nc.gpsimd.dma_scatter_add(
                    out_ap=out_flat,
                    in_ap=yE[:],
                    idxs_ap=dstw[:, e, :],
                    num_idxs=CAP,
                    num_idxs_reg=CAP,
                    elem_size=C,
                    single_packet=False,
                )

    nc.compile()
    return nc


def _host_consts():
    if "vals16" in _CACHE:
        return (_CACHE["vals16"], _CACHE["iotar"], _CACHE["iotae"],
                _CACHE["u128"], _CACHE["ones128"])
    # vals16: value v = 2t + k + 1 for slot (p, c) at scatter-in row
    # (i%128, i//128) of the [128, 64, TW] chunk layout, slot order i = p*64+c
    vals16 = np.zeros((128, NCOL), np.int16)
    i = np.arange(NSLOT)
    p, c = i // NCOL, i % NCOL
    t = (c // 2) * 128 + p
    k = c % 2
    vals16[i % 128, i // 128] = (2 * t + k + 1).astype(np.int16)
    pp_, bb_ = np.meshgrid(np.arange(128), np.arange(NBLK), indexing="ij")
    iotar = (bb_ * 128 + pp_).astype(np.float32)
    iotae = np.broadcast_to(np.arange(8, dtype=np.float32), (128, 8)).copy()
    u128 = np.triu(np.ones((128, 128), np.float32))
    ones128 = np.ones((128, 128), np.float32)
    _CACHE.update(vals16=vals16, iotar=iotar, iotae=iotae, u128=u128,
                  ones128=ones128)
    return vals16, iotar, iotae, u128, ones128


def _prep_in_maps(x, rms_weight, W1, b1, W2, b2, expert_ids):
    import ml_dtypes

    x = np.ascontiguousarray(np.asarray(x, dtype=np.float32))
    rmsw = np.asarray(rms_weight, dtype=np.float32)
    W1 = np.asarray(W1, dtype=np.float32)
    b1 = np.asarray(b1, dtype=np.float32)
    W2 = np.asarray(W2, dtype=np.float32)
    b2 = np.ascontiguousarray(np.asarray(b2, dtype=np.float32))
    ids = np.asarray(expert_ids).astype(np.int64)  # [B, T, K]

    if "w1h" not in _CACHE:
        w1f = rmsw[None, :, None] * W1  # fold rms weight into W1
        w1h = np.ascontiguousarray(
            w1f.reshape(E, 2, 128, WH).transpose(2, 0, 1, 3).reshape(128, E * 2 * WH)
        ).astype(ml_dtypes.bfloat16)
        w2h = np.ascontiguousarray(
            W2.reshape(E, 8, 128, C).transpose(2, 0, 1, 3).reshape(128, E * 8 * C)
        ).astype(ml_dtypes.bfloat16)
        b1h = np.ascontiguousarray(
            b1.reshape(E, 8, 128).transpose(2, 0, 1).reshape(128, E * 8)
        )
        _CACHE.update(w1h=w1h, w2h=w2h, b1h=b1h)
    w1h, w2h, b1h = _CACHE["w1h"], _CACHE["w2h"], _CACHE["b1h"]
    vals16, iotar, iotae, u128, ones128 = _host_consts()

    in_maps = []
    for b in range(B):
        ids_pc = (
            ids[b].reshape(32, 128, K).transpose(1, 0, 2).reshape(128, NCOL)
        ).astype(np.float32)
        in_maps.append({
            "x": x[b],
            "w1": w1h, "b1": b1h, "w2": w2h, "b2": b2,
            "ids": np.ascontiguousarray(ids_pc),
            "iotae": iotae,
            "u128": u128,
            "ones128": ones128,
            "vals": vals16,
            "iotar": iotar,
        })
    return in_maps


def run(inputs, trace=False, tmpdir=None):
    if "nc" not in _CACHE:
        _CACHE["nc"] = _build()
    nc = _CACHE["nc"]
    in_maps = _prep_in_maps(**inputs)
    kw = {}
    if trace:
        kw = dict(trace=True, tmpdir=tmpdir)
    res = bass_utils.run_bass_kernel_spmd(nc, in_maps, core_ids=list(range(B)), **kw)
    out = np.stack([res.results[i]["out"] for i in range(B)], axis=0)
    return out, res


def kernel(**inputs) -> np.ndarray:
    out, _ = run(inputs)
    return out



# revision 15
# speedup vs baseline: 1.5560x; 1.5560x over previous
"""Trainium2 Bass kernel for nn_ExpertsMLPBlock (MoE routing) — v4.

Problem (hardcoded):
  x          [8, 4096, 256] f32
  rms_weight [256]          f32
  W1         [8, 256, 1024] f32   b1 [8, 1024] f32
  W2         [8, 1024, 256] f32   b2 [8, 256]  f32
  expert_ids [8, 4096, 2]   int   (values 0..7)
  out        [8, 4096, 2, 256] f32

Sharding: data-parallel over B (batch row b -> core b), expert weights
replicated (pre-cast to bf16 on host, rms_weight folded into W1).
Routing metadata (per-expert slot lists) is computed on host (argsort of
expert_ids — pure control-plane; all tensor data stays on device).

Per-core algorithm:
  A. RMSNorm tokens -> xn (bf16) kept entirely in SBUF [128, 32, 256]
     (token t lives at partition t%128, free stripe t//128).
  D. Per expert e (prefetched):
     - SBUF-source transpose dma_gather: xgT[128, 2, CAP] bf16 straight
       from xn_sb using host-computed token lists (padding -> token 0).
     - h^T = gelu(W1^T xn^T + b1) ; y = h W2 + b2 -> yE[128, 9, 256] bf16
     - plain-write indirect scatter of the 1152 rows to out[slot] with
       bounds_check: padding rows (dst=100000) silently dropped.
  Output is bf16 on device; host casts back to f32.
"""

import numpy as np

import concourse.bacc as bacc
import concourse.bass as bass
import concourse.mybir as mybir
from concourse import bass_utils
from concourse.tile import TileContext
from concourse.alu_op_type import AluOpType

F32 = mybir.dt.float32
BF16 = mybir.dt.bfloat16
I32 = mybir.dt.int32
I16 = mybir.dt.int16

B, T, C, WH, E, K = 8, 4096, 256, 1024, 8, 2
NSLOT = T * K          # 8192 slots per core
RMS_EPS = 1.1920928955078125e-07
ACT_GELU = mybir.ActivationFunctionType.Gelu
ACT_SQRT = mybir.ActivationFunctionType.Sqrt
ACT_SQUARE = mybir.ActivationFunctionType.Square
ACT_COPY = mybir.ActivationFunctionType.Copy
OOB = 100000           # scatter dst for padding slots (> bounds_check)

_CACHE = {}


def _build(cap, dbg=False):
    nblk = cap // 128
    sw = cap // 16
    nc = bacc.Bacc("TRN2", target_bir_lowering=False, debug=False, num_devices=8)

    x_d = nc.dram_tensor("x", [T, C], F32, kind="ExternalInput")
    w1_d = nc.dram_tensor("w1", [128, E * 2 * WH], BF16, kind="ExternalInput")
    w2_d = nc.dram_tensor("w2", [128, E * 8 * C], BF16, kind="ExternalInput")
    b1_d = nc.dram_tensor("b1", [128, E * 8], F32, kind="ExternalInput")
    b2_d = nc.dram_tensor("b2", [E, C], F32, kind="ExternalInput")
    gidx_d = nc.dram_tensor("gidx", [128, E * sw], I16, kind="ExternalInput")
    doff_d = nc.dram_tensor("doff", [128, E * sw], I16, kind="ExternalInput")
    mask_d = nc.dram_tensor("mask", [128, E * nblk], F32, kind="ExternalInput")
    out_d = nc.dram_tensor("out", [NSLOT, C], BF16, kind="ExternalOutput")
    if dbg:
        xn_dbg = nc.dram_tensor("xn_dbg", [128, T // 128 * C], BF16,
                                kind="ExternalOutput")
        xg_dbg = nc.dram_tensor("xg_dbg", [128, 2 * cap], BF16,
                                kind="ExternalOutput")
        ye_dbg = nc.dram_tensor("ye_dbg", [128, nblk * C], BF16,
                                kind="ExternalOutput")

    with TileContext(nc) as tc:
        with (
            tc.tile_pool(name="const", bufs=1) as constp,
            tc.tile_pool(name="norm", bufs=3) as normp,
            tc.tile_pool(name="act", bufs=4) as actp,
            tc.tile_pool(name="hpool", bufs=3) as hp,
            tc.tile_pool(name="ypool", bufs=2) as yp,
            tc.tile_pool(name="psum", bufs=4, space="PSUM") as pp,
            tc.tile_pool(name="psumy", bufs=2, space="PSUM") as ppy,
        ):
            # ---- constants / weights ----
            # x tiles go on the sync (SP) HWDGE ring; weights/idx on the
            # scalar (ACT) ring so the norm pipeline is never starved.
            gidxt = constp.tile([128, E, sw], I16)
            nc.scalar.dma_start(
                out=gidxt[:], in_=gidx_d.ap().rearrange("p (e s) -> p e s", e=E)
            )
            dofft = constp.tile([128, E, sw], I16)
            nc.scalar.dma_start(
                out=dofft[:], in_=doff_d.ap().rearrange("p (e s) -> p e s", e=E)
            )
            maskt = constp.tile([128, E, nblk], F32)
            nc.scalar.dma_start(
                out=maskt[:], in_=mask_d.ap().rearrange("p (e b) -> p e b", e=E)
            )
            # zero-init out (scatter_add accumulates into it)
            zout = constp.tile([128, C], BF16)
            nc.vector.memset(zout[:], 0.0)
            nc.scalar.dma_start(
                out=out_d.ap().rearrange("(a p) c -> p a c", p=128),
                in_=zout[:].rearrange("p (o c) -> p o c", o=1).to_broadcast(
                    [128, NSLOT // 128, C]
                ),
            )
            w1sb = constp.tile([128, E, 2, WH], BF16)
            nc.scalar.dma_start(
                out=w1sb[:], in_=w1_d.ap().rearrange("p (e c w) -> p e c w", e=E, c=2)
            )
            w2sb = constp.tile([128, E, 8, C], BF16)
            nc.scalar.dma_start(
                out=w2sb[:], in_=w2_d.ap().rearrange("p (e w c) -> p e w c", e=E, w=8)
            )
            b1sb = constp.tile([128, E, 8], F32)
            nc.scalar.dma_start(
                out=b1sb[:], in_=b1_d.ap().rearrange("p (e w) -> p e w", e=E)
            )
            b2sb = constp.tile([128, E, C], F32)
            nc.scalar.dma_start(
                out=b2sb[:],
                in_=b2_d.ap().rearrange("(o e) c -> o e c", o=1).to_broadcast([128, E, C]),
            )

            # ---- phase A: RMSNorm -> xn_sb (bf16, stays in SBUF) ----
            # xn_sb[p, j, :] = normed token j*128 + p
            xn_sb = constp.tile([128, T // 128, C], BF16)
            AW = 4
            for m in range(T // (128 * AW)):
                xt = normp.tile([128, AW, C], F32, tag="xt")
                nc.sync.dma_start(
                    out=xt[:],
                    in_=x_d[m * 128 * AW:(m + 1) * 128 * AW, :].rearrange(
                        "(a p) c -> p a c", p=128
                    ),
                )
                ms = normp.tile([128, AW], F32, tag="ms")
                sq = normp.tile([128, AW, C], BF16, tag="sq")
                for a in range(AW):
                    nc.scalar.activation(
                        sq[:, a, :], xt[:, a, :], ACT_SQUARE,
                        accum_out=ms[:, a:a + 1],
                    )
                ms2 = normp.tile([128, AW], F32, tag="ms2")
                nc.vector.tensor_scalar(
                    out=ms2[:], in0=ms[:], scalar1=1.0 / C, scalar2=RMS_EPS,
                    op0=AluOpType.mult, op1=AluOpType.add,
                )
                sr = normp.tile([128, AW], F32, tag="sr")
                nc.scalar.activation(sr[:], ms2[:], ACT_SQRT)
                rstd = normp.tile([128, AW], F32, tag="rstd")
                nc.vector.reciprocal(rstd[:], sr[:])
                for a in range(AW):
                    nc.scalar.activation(
                        xn_sb[:, m * AW + a, :], xt[:, a, :], ACT_COPY,
                        scale=rstd[:, a:a + 1],
                    )

            # ---- phase D: per-expert MLP ----
            t5_sizes = [512] * (cap // 512) + ([cap % 512] if cap % 512 else [])

            def issue_gather(e):
                xg = actp.tile([128, 2, cap], BF16, tag="xgT")
                nc.gpsimd.dma_gather(
                    out_ap=xg[:],
                    in_ap=xn_sb[:],
                    idxs_ap=gidxt[:, e, :],
                    num_idxs=cap,
                    num_idxs_reg=cap,
                    elem_size=C,
                    transpose=True,
                    single_packet=False,
                    sbuf_tokens_per_rank=128,
                    sbuf_free_dim_per_rank=C * 2,
                )
                return xg

            PFD = 3
            xg_tiles = [issue_gather(e) for e in range(PFD)]
            for e in range(E):
                xgT = xg_tiles[e]

                yE = yp.tile([128, nblk, C], BF16, tag="yE")
                hTs = []
                for t5, ts in enumerate(t5_sizes):
                    off = t5 * 512
                    hT = hp.tile([128, 8, 512], BF16, tag="hT")
                    hTs.append(hT)
                    for wc in range(8):
                        hps = pp.tile([128, 512], F32, tag="hps")
                        for cc in range(2):
                            nc.tensor.matmul(
                                hps[:, :ts],
                                lhsT=w1sb[:, e, cc, wc * 128:(wc + 1) * 128],
                                rhs=xgT[:, cc, off:off + ts],
                                start=(cc == 0), stop=(cc == 1),
                            )
                        nc.scalar.activation(
                            hT[:, wc, :ts], hps[:, :ts], ACT_GELU,
                            bias=b1sb[:, e, wc:wc + 1],
                        )
                for t5, ts in enumerate(t5_sizes):
                    hT = hTs[t5]
                    for tb in range(ts // 128):
                        blk = t5 * 4 + tb
                        yps = ppy.tile([128, C], F32, tag="yps")
                        for wc in range(8):
                            nc.tensor.matmul(
                                yps[:],
                                lhsT=hT[:, wc, tb * 128:(tb + 1) * 128],
                                rhs=w2sb[:, e, wc, :],
                                start=(wc == 0), stop=(wc == 7),
                            )
                        nc.vector.tensor_add(yE[:, blk, :], yps[:], b2sb[:, e, :])
                        nc.vector.tensor_tensor(
                            out=yE[:, blk, :],
                            in0=yE[:, blk, :],
                            in1=maskt[:, e, blk:blk + 1].to_broadcast([128, C]),
                            op=AluOpType.mult,
                        )

                if e + PFD < E:
                    xg_tiles.append(issue_gather(e + PFD))
                nc.gpsimd.dma_scatter_add(
                    out_ap=out_d.ap(),
                    in_ap=yE[:],
                    idxs_ap=dofft[:, e, :],
                    num_idxs=cap,
                    num_idxs_reg=cap,
                    elem_size=C,
                    single_packet=False,
                )
                if dbg and e == 0:
                    nc.sync.dma_start(
                        out=xg_dbg.ap().rearrange("p (c s) -> p c s", c=2),
                        in_=xgT[:],
                    )
                    nc.sync.dma_start(
                        out=ye_dbg.ap().rearrange("p (b c) -> p b c", b=nblk),
                        in_=yE[:],
                    )
            if dbg:
                nc.sync.dma_start(
                    out=xn_dbg.ap().rearrange("p (a c) -> p a c", a=T // 128),
                    in_=xn_sb[:],
                )

    nc.compile()
    return nc


def _route(ids, cap):
    """ids [T, K] -> (gidx128, doff128 [128, E*SW] i16, counts [E] i32).

    Slot lists per expert (stable sort order), trailing -1 padding (the
    gather/scatter ucode trims trailing negatives). Both idx tensors use the
    wrap-16 layout (idx i at [i%16, i//16], replicated to 128 partitions).
    """
    sw = cap // 16
    nblk = cap // 128
    e_flat = ids.reshape(-1)
    order = np.argsort(e_flat, kind="stable")
    counts = np.bincount(e_flat, minlength=E)
    gidx = np.zeros((E, cap), np.int16)
    doff = np.zeros((E, cap), np.int16)
    pos = 0
    for e in range(E):
        n = min(counts[e], cap)
        sl = order[pos:pos + counts[e]][:n]
        gidx[e, :n] = (sl >> 1).astype(np.int16)
        doff[e, :n] = sl.astype(np.int16)
        pos += counts[e]

    def wrap128(a):
        a16 = a.reshape(E, sw, 16).transpose(2, 0, 1)     # [16, E, SW]
        return np.ascontiguousarray(np.tile(a16, (8, 1, 1)).reshape(128, E * sw))

    # mask[p, e, blk] = 1 if slot blk*128+p is real for expert e
    cnt = np.minimum(counts, cap)
    p = np.arange(128)[:, None, None]
    blk = np.arange(nblk)[None, None, :]
    mask = ((blk * 128 + p) < cnt[None, :, None]).astype(np.float32)  # [128, E, nblk]
    mask = np.ascontiguousarray(mask.reshape(128, E * nblk))
    return wrap128(gidx), wrap128(doff), mask


def _prep_weights(rms_weight, W1, b1, W2, b2):
    import ml_dtypes

    rmsw = np.asarray(rms_weight, dtype=np.float32)
    W1 = np.asarray(W1, dtype=np.float32)
    b1 = np.asarray(b1, dtype=np.float32)
    W2 = np.asarray(W2, dtype=np.float32)
    b2 = np.ascontiguousarray(np.asarray(b2, dtype=np.float32))

    w1f = rmsw[None, :, None] * W1  # fold rms weight into W1
    w1h = np.ascontiguousarray(
        w1f.reshape(E, 2, 128, WH).transpose(2, 0, 1, 3).reshape(128, E * 2 * WH)
    ).astype(ml_dtypes.bfloat16)
    w2h = np.ascontiguousarray(
        W2.reshape(E, 8, 128, C).transpose(2, 0, 1, 3).reshape(128, E * 8 * C)
    ).astype(ml_dtypes.bfloat16)
    b1h = np.ascontiguousarray(
        b1.reshape(E, 8, 128).transpose(2, 0, 1).reshape(128, E * 8)
    )
    return w1h, b1h, w2h, b2


def _prep_in_maps(x, rms_weight, W1, b1, W2, b2, expert_ids, cap):
    x = np.ascontiguousarray(np.asarray(x, dtype=np.float32))
    ids = np.asarray(expert_ids).astype(np.int64)  # [B, T, K]

    if "w1h" not in _CACHE:
        _CACHE["w1h"], _CACHE["b1h"], _CACHE["w2h"], _CACHE["b2h"] = _prep_weights(
            rms_weight, W1, b1, W2, b2
        )
    w1h, b1h, w2h, b2h = (_CACHE[k] for k in ("w1h", "b1h", "w2h", "b2h"))

    in_maps = []
    for b in range(B):
        gidx128, doff128, mask128 = _route(ids[b], cap)
        in_maps.append({
            "x": x[b],
            "w1": w1h, "b1": b1h, "w2": w2h, "b2": b2h,
            "gidx": gidx128,
            "doff": doff128,
            "mask": mask128,
        })
    return in_maps


def run(inputs, trace=False, tmpdir=None):
    ids = np.asarray(inputs["expert_ids"])
    maxc = max(
        int(np.bincount(ids[b].reshape(-1), minlength=E).max()) for b in range(B)
    )
    cap = max(1152, -(-maxc // 128) * 128)
    if _CACHE.get("cap") != cap:
        _CACHE["nc"] = _build(cap)
        _CACHE["cap"] = cap
    nc = _CACHE["nc"]
    in_maps = _prep_in_maps(**inputs, cap=cap)
    kw = {}
    if trace:
        kw = dict(trace=True, tmpdir=tmpdir)
    res = bass_utils.run_bass_kernel_spmd(nc, in_maps, core_ids=list(range(B)), **kw)
    out = np.stack(
        [res.results[i]["out"].astype(np.float32) for i in range(B)], axis=0
    )
    return out.reshape(B, T, K, C), res


def kernel(**inputs) -> np.ndarray:
    out, _ = run(inputs)
    return out
